# revision 15
# baseline (speedup 1.0000x reference)
"""Trainium2 Bass kernel for grouped-expert 3-layer MLP (MoE, known covariance).

Computes, for x[B, E, DIN] and per-expert weights:
    h1 = relu(x[:,e] @ W1[e] + b1[e])      # [B, H]
    h2 = relu(h1 @ W2[e] + b2[e])          # [B, H]
    o  = h2 @ W3[e] + b3[e]                # [B, DOUT]
    out = sum_e o                          # [B, DOUT]

Sharding: data-parallel over batch across 8 NeuronCores (B=8192 -> 1024/core).
Weights are replicated to every core; no collectives needed.

Per-core layout strategy (feature-major activations):
  x is PE-transposed on-chip into xT [DIN, batch] tiles; all three layers run
  with the weight panel as the stationary operand and activations streaming
  feature-major (L1/L2 in float32r, 1 PE cycle/row at N>=256).  The final
  oT = sum_e W3[e].T @ h2[e] runs in bf16 and accumulates in a single PSUM
  bank across the whole expert loop, with the two batch tiles column-packed
  into partitions 0-63 / 64-127: M=64 matmuls to distinct col-groups run
  CONCURRENTLY on the PE's 32x32 sub-arrays (~2x on layer 3).

Software pipeline (per expert round e, emission order):
    x-DMA(e+2) | w-DMA(e+2) | T(e+1) | L1(e) | L2(e,bt0) | L3(e-1) | L2(e,bt1)
  Transposes run one round ahead and layer 3 is delayed one round, so every
  PSUM evacuation (PSUM->SBUF on ACT or DVE, ~600ns each, the critical
  latency) has several microseconds of slack before its PE consumer.
"""

import os
from contextlib import ExitStack

import bass_rust
import numpy as np

import concourse.bass as bass
import concourse.tile as tile
from concourse import bacc, mybir
from concourse.bass_utils import run_bass_kernel_spmd
from concourse.masks import make_identity

E, DIN, H, DOUT = 16, 128, 512, 64
B_FULL = 8192
N_CORES = 8
HB = H // 128  # 4 h-blocks
F32 = mybir.dt.float32
FR = mybir.dt.float32r
BF = mybir.dt.bfloat16


def build_nc(bloc=B_FULL // N_CORES, nb=512):
    """Build the per-core Bass program. bloc = local batch, nb = batch tile."""
    nbt = bloc // nb
    nt = nb // 128  # 128-row chunks per batch tile
    assert bloc % nb == 0 and nb % 128 == 0
    assert nbt == 2, "col-packed L3 assumes exactly 2 batch tiles"

    nc = bacc.Bacc("TRN2", target_bir_lowering=False, debug=False)

    x = nc.dram_tensor("x", [bloc, E, DIN], F32, kind="ExternalInput")
    W1 = nc.dram_tensor("W1", [E, DIN, H], F32, kind="ExternalInput")
    b1 = nc.dram_tensor("b1", [E, H], F32, kind="ExternalInput")
    W2 = nc.dram_tensor("W2", [E, H, H], F32, kind="ExternalInput")
    b2 = nc.dram_tensor("b2", [E, H], F32, kind="ExternalInput")
    W3 = nc.dram_tensor("W3", [E, H, DOUT], F32, kind="ExternalInput")
    b3 = nc.dram_tensor("b3", [E, DOUT], F32, kind="ExternalInput")
    out = nc.dram_tensor("out", [bloc, DOUT], F32, kind="ExternalOutput")

    RELU = mybir.ActivationFunctionType.Relu
    IDENT = mybir.ActivationFunctionType.Identity
    ADD = mybir.AluOpType.add
    MAX = mybir.AluOpType.max

    with tile.TileContext(nc) as tc, ExitStack() as ctx:
        consts = ctx.enter_context(tc.tile_pool(name="consts", bufs=1))
        w1p = ctx.enter_context(tc.tile_pool(name="w1p", bufs=2))
        w2p = ctx.enter_context(tc.tile_pool(name="w2p", bufs=2))
        w3p = ctx.enter_context(tc.tile_pool(name="w3p", bufs=2))
        xp = ctx.enter_context(tc.tile_pool(name="xp", bufs=6))
        xtp = ctx.enter_context(tc.tile_pool(name="xtp", bufs=4))
        h1p = ctx.enter_context(tc.tile_pool(name="h1p", bufs=2))
        h2p = ctx.enter_context(tc.tile_pool(name="h2p", bufs=2))
        obp = ctx.enter_context(tc.tile_pool(name="obp", bufs=2))
        pxt = ctx.enter_context(tc.tile_pool(name="pxt", bufs=2, space="PSUM"))
        p1p = ctx.enter_context(tc.tile_pool(name="p1p", bufs=2, space="PSUM"))
        p2p = ctx.enter_context(tc.tile_pool(name="p2p", bufs=3, space="PSUM"))
        pop = ctx.enter_context(tc.tile_pool(name="pop", bufs=1, space="PSUM"))

        def xdma(e):
            # x tiles ride the (otherwise idle) sync HWDGE queue so they never
            # queue behind the big weight DMAs on gpsimd.
            tiles = []
            for bt in range(nbt):
                xin = xp.tile([128, nt, DIN], FR, tag="xin")
                nc.sync.dma_start(
                    out=xin,
                    in_=x[bt * nb : (bt + 1) * nb, e, :]
                    .rearrange("(t p) d -> p t d", p=128)
                    .bitcast(FR),
                )
                tiles.append(xin)
            return tiles

        def wdma12(e):
            w1t = w1p.tile([DIN, H], FR, tag="w1")
            nc.gpsimd.dma_start(out=w1t, in_=W1[e])
            w2t = w2p.tile([128, HB, H], BF, tag="w2")
            for whb in range(HB):
                nc.gpsimd.dma_start(
                    out=w2t[:, whb, :], in_=W2[e, whb * 128 : (whb + 1) * 128, :]
                )
            return w1t, w2t

        def wdma3(e):
            w3t = w3p.tile([128, HB, DOUT], BF, tag="w3")
            nc.gpsimd.dma_start(out=w3t, in_=W3[e].rearrange("(hb p) o -> p hb o", p=128))
            return w3t

        # ---- prologue: identity + small bias DMAs first so the PE can start
        # its bias transposes early; then the big expert-0 weight/x DMAs ----
        ident = consts.tile([128, 128], F32)
        make_identity(nc, ident)
        b1n = consts.tile([E, H], F32)
        nc.sync.dma_start(out=b1n, in_=b1[:, :])

        x_tiles = {0: xdma(0)}
        w12 = {0: wdma12(0)}
        w3s = {0: wdma3(0)}

        identr = consts.tile([128, 128], FR)
        nc.scalar.copy(identr, ident)

        # biases: load in natural layout, PE-transpose so the per-feature bias
        # lands on partitions: b1s[p, hb*E + e] = b1[e, hb*128 + p]
        b2n = consts.tile([E, H], F32)
        nc.sync.dma_start(out=b2n, in_=b2[:, :])
        b3n = consts.tile([E, DOUT], F32)
        nc.sync.dma_start(out=b3n, in_=b3[:, :])
        b1s = consts.tile([128, HB * E], F32)
        b2s = consts.tile([128, HB * E], F32)
        for bn, bs in ((b1n, b1s), (b2n, b2s)):
            pb = pxt.tile([128, HB * E], F32, tag="pxt", name="pb")
            for hb in range(HB):
                nc.tensor.transpose(
                    pb[:, hb * E : (hb + 1) * E],
                    bn[:, hb * 128 : (hb + 1) * 128],
                    ident[:E, :E],
                )
            nc.vector.tensor_copy(bs, pb)
        # b3 summed over experts, then replicated onto partitions 64-127 with a
        # small SBUF->SBUF DMA (transpose-mode matmuls must output to PSUM
        # partition 0) so both halves of the col-packed bank get a bias.
        pb3 = pxt.tile([128, E], F32, tag="pxt", name="pb3")
        nc.tensor.transpose(pb3[0:DOUT, :], b3n, ident[:E, :E])
        b3s = consts.tile([128, E], F32)
        nc.vector.tensor_copy(b3s[0:DOUT, :], pb3[0:DOUT, :])
        b3sumA = consts.tile([128, 1], F32)
        nc.vector.reduce_sum(
            b3sumA[0:DOUT, :], b3s[0:DOUT, :], axis=bass_rust.AxisListType.X
        )
        b3sum = consts.tile([128, 1], F32)
        nc.gpsimd.dma_start(out=b3sum[0:DOUT, :], in_=b3sumA[0:DOUT, :])
        nc.gpsimd.dma_start(out=b3sum[DOUT : 2 * DOUT, :], in_=b3sumA[0:DOUT, :])

        # second-round x prefetch (weights are single-round prefetched: bufs=2)
        x_tiles[1] = xdma(1)

        # PSUM accumulator for the expert-summed output: ONE bank, batch tile 0
        # on partitions 0-63, batch tile 1 on partitions 64-127.
        po = pop.tile([128, nb], F32, tag="po", name="po")

        def do_transposes(e):
            # PE transposes only; the PSUM->SBUF copies are emitted later (after
            # the L1 evacs) so the ACT/DVE queues run PE-gating evacs first.
            pxts = []
            for bt in range(nbt):
                pxt_t = pxt.tile([DIN, nb], FR, tag="pxt")
                for t in range(nt):
                    nc.tensor.transpose(
                        pxt_t[:, t * 128 : (t + 1) * 128], x_tiles[e][bt][:, t, :], identr
                    )
                pxts.append(pxt_t)
            del x_tiles[e]
            return pxts

        def evac_transposes(pxts):
            xts = []
            for bt in range(nbt):
                xt = xtp.tile([DIN, nb], FR, tag="xt")
                if bt == 0:
                    nc.scalar.copy(xt, pxts[bt])
                else:
                    nc.vector.tensor_copy(xt, pxts[bt])
                xts.append(xt)
            return xts

        def do_l3(h2s, w3t, e):
            for gb in range(HB):
                for bt in range(nbt):
                    nc.tensor.matmul(
                        po[bt * DOUT : (bt + 1) * DOUT, :],
                        w3t[:, gb, :],
                        h2s[bt, gb],
                        start=(e == 0 and gb == 0),
                        stop=(e == E - 1 and gb == HB - 1),
                    )

        def do_l3_bt(h2s, w3t, e, bt):
            for gb in range(HB):
                nc.tensor.matmul(
                    po[bt * DOUT : (bt + 1) * DOUT, :],
                    w3t[:, gb, :],
                    h2s[bt, gb],
                    start=(e == 0 and gb == 0),
                    stop=(e == E - 1 and gb == HB - 1),
                )

        def epilogue_bt(bt):
            # po partitions [0:64] = batch tile 0 (transposed), [64:128] = tile 1
            rows = slice(bt * DOUT, (bt + 1) * DOUT)
            ob = obp.tile([128, nb], F32, tag=f"ob{bt}")
            if bt == 0:
                nc.scalar.activation(ob[rows, :], po[rows, :], IDENT, bias=b3sum[rows, :])
            else:
                nc.vector.tensor_scalar_add(ob[rows, :], po[rows, :], b3sum[rows, :])
            pot = pxt.tile([128, nt * DOUT], F32, tag="pxt")
            for t in range(nt):
                if bt == 0:
                    nc.tensor.transpose(
                        pot[:, t * DOUT : (t + 1) * DOUT],
                        ob[rows, t * 128 : (t + 1) * 128],
                        ident[:DOUT, :DOUT],
                    )
                else:
                    # transpose-mode must output to partition 0 with inputs at
                    # partition 0; for the upper half use a plain matmul against
                    # identity (contraction over partitions 64-127) instead.
                    nc.tensor.matmul(
                        pot[:, t * DOUT : (t + 1) * DOUT],
                        ob[rows, t * 128 : (t + 1) * 128],
                        ident[rows, rows],
                        start=True, stop=True,
                    )
            obt = obp.tile([128, nt * DOUT], F32, tag=f"obt{bt}")
            if bt == 0:
                nc.vector.tensor_copy(obt, pot)
            else:
                nc.scalar.copy(obt, pot)
            nc.sync.dma_start(
                out=out[bt * nb : (bt + 1) * nb, :].rearrange("(t p) o -> p t o", p=128),
                in_=obt.rearrange("p (t o) -> p t o", o=DOUT),
            )

        xt_cur = evac_transposes(do_transposes(0))
        prev = None  # (h2 tiles, w3 tile) for the delayed L3 of expert e-1
        for e in range(E):
            w1t, w2t = w12.pop(e)
            w3t = w3s.pop(e)
            if e + 2 < E:
                x_tiles[e + 2] = xdma(e + 2)
            if e + 1 < E:
                w12[e + 1] = wdma12(e + 1)

            # ---- transpose phase for the NEXT round ----
            pxts_next = do_transposes(e + 1) if e + 1 < E else None

            # ---- layer 1 (evacs alternate ACT/DVE, whole tiles) ----
            h1s = {}
            for bt in range(nbt):
                for hb in range(HB):
                    ps = p1p.tile([128, nb], F32, tag="p1")
                    nc.tensor.matmul(
                        ps, w1t[:, hb * 128 : (hb + 1) * 128], xt_cur[bt],
                        start=True, stop=True,
                    )
                    ht = h1p.tile([128, nb], BF, tag=f"h1_{bt}_{hb}")
                    bias = b1s[:, hb * E + e : hb * E + e + 1]
                    nh = nb // 2
                    nc.vector.tensor_scalar(ht[:, :nh], ps[:, :nh], bias, 0.0, ADD, MAX)
                    nc.scalar.activation(ht[:, nh:], ps[:, nh:], RELU, bias=bias)
                    h1s[bt, hb] = ht

            # ---- layer 2 bt0, then delayed L3(e-1), then layer 2 bt1 ----
            h2s = {}

            def l2(bt):
                for gb in range(HB):
                    ps = p2p.tile([128, nb], F32, tag="p2")
                    for hb in range(HB):
                        nc.tensor.matmul(
                            ps,
                            w2t[:, hb, gb * 128 : (gb + 1) * 128],
                            h1s[bt, hb],
                            start=(hb == 0),
                            stop=(hb == HB - 1),
                        )
                    ht = h2p.tile([128, nb], BF, tag=f"h2_{bt}_{gb}")
                    bias = b2s[:, gb * E + e : gb * E + e + 1]
                    nh = nb // 2
                    nc.scalar.activation(ht[:, :nh], ps[:, :nh], RELU, bias=bias)
                    nc.vector.tensor_scalar(ht[:, nh:], ps[:, nh:], bias, 0.0, ADD, MAX)
                    h2s[bt, gb] = ht

            xt_next = evac_transposes(pxts_next) if pxts_next is not None else None
            if prev is not None:
                do_l3(prev[0], prev[1], e - 1)
            if e + 1 < E:
                w3s[e + 1] = wdma3(e + 1)
            l2(0)
            if e == E - 1:
                do_l3_bt(h2s, w3t, e, 0)
                epilogue_bt(0)
            l2(1)

            prev = (h2s, w3t)
            xt_cur = xt_next

        do_l3_bt(prev[0], prev[1], E - 1, 1)
        epilogue_bt(1)

    nc.compile()
    return nc


_NC_CACHE = {}


def _get_nc():
    if "nc" not in _NC_CACHE:
        _NC_CACHE["nc"] = build_nc()
    return _NC_CACHE["nc"]


def kernel(x, W1, b1, W2, b2, W3, b3):
    x = np.ascontiguousarray(np.asarray(x, dtype=np.float32))
    ws = {
        "W1": np.ascontiguousarray(np.asarray(W1, dtype=np.float32)),
        "b1": np.ascontiguousarray(np.asarray(b1, dtype=np.float32)),
        "W2": np.ascontiguousarray(np.asarray(W2, dtype=np.float32)),
        "b2": np.ascontiguousarray(np.asarray(b2, dtype=np.float32)),
        "W3": np.ascontiguousarray(np.asarray(W3, dtype=np.float32)),
        "b3": np.ascontiguousarray(np.asarray(b3, dtype=np.float32)),
    }
    nc = _get_nc()
    shards = np.split(x, N_CORES, axis=0)
    in_maps = [{"x": np.ascontiguousarray(s), **ws} for s in shards]
    trace = bool(int(os.environ.get("KERNEL_TRACE", "0")))
    kwargs = {}
    if trace and os.environ.get("KERNEL_TRACE_DIR"):
        kwargs["tmpdir"] = os.environ["KERNEL_TRACE_DIR"]
    res = run_bass_kernel_spmd(nc, in_maps, list(range(N_CORES)), trace=trace, **kwargs)
    if trace:
        kernel.last_results = res
    return np.concatenate([res.results[c]["out"] for c in range(N_CORES)], axis=0)
